# revision 1
# baseline (speedup 1.0000x reference)
"""Trainium2 Bass kernel for NeuralSumProductModel (LDPC sum-product decoder).

Contract: kernel(**inputs) takes FULL inputs (llr [512,8192] f32,
var_index [24576] i32, chk_index [24576] i32) and returns the FULL
output [5, 512, 8192] f32, matching reference.reference().

Design (per NeuronCore, batch sharded 512 -> 8 x 64):
  - partitions = (edge-half h, batch b): p = h*64 + b
  - edges in check-major order (sorted by check, 6 per check); half h owns
    checks [h*2048,(h+1)*2048) = edge cols [h*12288,(h+1)*12288)
  - one big SBUF gather TABLE [128, 45056] f32:
      [locA 0:12288 | foreign 12288:24576 | locB 24576:36864 | x 36864:45056]
    ext state ping-pongs between locA/locB by iteration parity so gathers of
    the old state never conflict with writes of the new state. 'foreign' is
    cross-filled by DMA from the partner partition half each iteration.
  - var-side ops are GPSIMD ap_gather's: msg_e = xs_e + ext[sib1] + ext[sib2]
    (siblings share e's variable), out_v = x_v + sum of ext at v's 3 edges.
  - check-side ops are strided free-axis DVE/ACT ops over groups of 6.
  - magnitude via phi involution: ext_mag = -ln(max(tanh(-d/2), TCLIP)),
    exactly 2*atanh(min(e^d, 1-1e-7)); sign via float sign-product tree.
"""

import os
import sys

import numpy as np

for _p in ("/opt/trn_rl_repo", "/root/.axon_site/_ro/trn_rl_repo"):
    if os.path.isdir(_p) and _p not in sys.path:
        sys.path.insert(0, _p)

N_VAR, N_CHK, DV, DC = 8192, 4096, 3, 6
E = N_VAR * DV  # 24576
BATCH, N_ITER, N_CORES = 512, 5, 8
BC = BATCH // N_CORES           # 64 batch rows per core
HE = E // 2                     # 12288 edge cols per half
HC = N_CHK // 2                 # 2048 checks per half
HV = N_VAR // 2                 # 4096 vars per half
N_ECH = 16                      # check chunks per iteration
ECH = HE // N_ECH               # 768 edge cols per chunk
CCH = ECH // DC                 # 128 checks per chunk
N_VCH = 16                      # var chunks
VCH = HV // N_VCH               # 256 vars per chunk
T_LOCA, T_FOR, T_LOCB, T_X = 0, HE, 2 * HE, 3 * HE
T_COLS = 3 * HE + N_VAR         # 45056
WIN = 2 * HE                    # 24576-col sib gather window

EPS = 1e-12
_C = np.float32(1.0) - np.float32(1e-7)
TCLIP = float(np.float32((np.float32(1.0) - _C) / (np.float32(1.0) + _C)))

_CACHE = {}
_LAST_RESULTS = None


def _wrap(stream):
    """Pack an unwrapped per-core index stream [8, n] -> wrapped [128, n//16].

    ap_gather unwraps core k's indices as unwrapped[s*16+p] = tile[16k+p, s].
    """
    st = np.asarray(stream, np.int16)
    ncore, n = st.shape
    assert n % 16 == 0
    out = np.zeros((16 * ncore, n // 16), np.int16)
    for k in range(ncore):
        out[16 * k:16 * (k + 1), :] = st[k].reshape(n // 16, 16).T
    return out


def _build_indices(vi, ci):
    """Host-side graph preprocessing. Returns dict of wrapped index planes."""
    order = np.argsort(ci, kind="stable")          # check-major edge list
    cm_var = vi[order].astype(np.int64)            # var of each cm edge
    pos_of_edge = np.empty(E, np.int64)
    pos_of_edge[order] = np.arange(E)
    edges_of_var = np.argsort(vi, kind="stable").reshape(N_VAR, DV)
    pos_var = pos_of_edge[edges_of_var]            # [N_VAR, 3] cm positions

    half_of_pos = pos_var // HE                    # [N_VAR, 3]

    def rel(p, H, parity):
        # relative coord of global cm position p within the sib window of
        # `parity` (0 = A window [0:24576), 1 = B window [12288:36864)),
        # as seen from a partition in half H.
        same = (p // HE) == H
        if parity == 0:
            return (p % HE) + HE * (~same)
        return (p % HE) + HE * same

    planes = {}
    # sibling + x index streams, per half
    for parity in (0, 1):
        s1 = np.zeros((2, HE), np.int64)
        s2 = np.zeros((2, HE), np.int64)
        for H in (0, 1):
            jj = np.arange(H * HE, (H + 1) * HE)
            v = cm_var[jj]                          # [HE]
            pv = pos_var[v]                         # [HE, 3]
            # sibling positions: the 2 of pv != jj, kept in slot order
            mask = pv != jj[:, None]
            sib = pv[mask].reshape(HE, 2)
            s1[H] = rel(sib[:, 0], H, parity)
            s2[H] = rel(sib[:, 1], H, parity)
        planes[f"s1{'ab'[parity]}"] = _wrap(
            np.concatenate([np.repeat(s1[0][None], 4, 0),
                            np.repeat(s1[1][None], 4, 0)]))
        planes[f"s2{'ab'[parity]}"] = _wrap(
            np.concatenate([np.repeat(s2[0][None], 4, 0),
                            np.repeat(s2[1][None], 4, 0)]))
    xi = np.zeros((2, HE), np.int64)
    for H in (0, 1):
        xi[H] = cm_var[np.arange(H * HE, (H + 1) * HE)]
    planes["xi"] = _wrap(np.concatenate([np.repeat(xi[0][None], 4, 0),
                                         np.repeat(xi[1][None], 4, 0)]))

    # out gathers: var v (local to half H) -> its 3 edge positions
    for parity in (0, 1):
        for s in range(DV):
            vg = np.zeros((2, HV), np.int64)
            for H in (0, 1):
                vids = np.arange(H * HV, (H + 1) * HV)
                vg[H] = rel(pos_var[vids, s], H, parity)
            planes[f"vg{s}{'ab'[parity]}"] = _wrap(
                np.concatenate([np.repeat(vg[0][None], 4, 0),
                                np.repeat(vg[1][None], 4, 0)]))
    return planes


def _build_bass():
    import concourse.bass as bass
    import concourse.tile as tile
    from concourse import bacc, mybir
    from contextlib import ExitStack

    dt = mybir.dt
    F32, I16 = dt.float32, dt.int16
    ALU = mybir.AluOpType
    ACT = mybir.ActivationFunctionType
    AX = mybir.AxisListType

    nc = bacc.Bacc("TRN2", target_bir_lowering=False, debug=False)

    llr_d = nc.dram_tensor("llr", [BC, N_VAR], F32, kind="ExternalInput").ap()
    idx_d = {}
    for nm in ("s1a", "s1b", "s2a", "s2b", "xi"):
        idx_d[nm] = nc.dram_tensor(nm, [128, HE // 16], I16,
                                   kind="ExternalInput").ap()
    for parity in (0, 1):
        for s in range(DV):
            nm = f"vg{s}{'ab'[parity]}"
            idx_d[nm] = nc.dram_tensor(nm, [128, HV // 16], I16,
                                       kind="ExternalInput").ap()
    out_d = nc.dram_tensor("out", [N_ITER, BC, N_VAR], F32,
                           kind="ExternalOutput").ap()

    with tile.TileContext(nc) as tc, ExitStack() as ctx:
        big = ctx.enter_context(tc.tile_pool(name="big", bufs=1))
        wp = ctx.enter_context(tc.tile_pool(name="wp", bufs=1))
        pp = ctx.enter_context(tc.tile_pool(name="pp", bufs=1, space="PSUM"))

        table = big.tile([128, T_COLS], F32, tag="table")
        # persistent smalls: csum|cp1|cp|p3|eps packed in one 4KB tile
        sm = big.tile([128, 772], F32, tag="smalls")
        sm_csum = sm[:, 0:CCH]
        sm_cp1 = sm[:, CCH:2 * CCH]
        sm_cp = sm[:, 2 * CCH:3 * CCH]
        sm_p3 = sm[:, 3 * CCH:6 * CCH]
        sm_eps = sm[:, 768:769]
        nc.vector.memset(sm_eps, EPS)

        # load x region (duplicated across halves)
        nc.sync.dma_start(table[0:64, T_X:T_X + N_VAR], llr_d[:, :])
        nc.sync.dma_start(table[64:128, T_X:T_X + N_VAR], llr_d[:, :])

        xwin = table[:, T_X:T_X + N_VAR]
        IC = ECH // 16          # wrapped idx cols per check chunk (48)
        IVC = VCH // 16         # wrapped idx cols per var chunk (16)

        for it in range(N_ITER):
            side = it % 2
            wr = T_LOCA if side == 0 else T_LOCB
            wloc = table[:, wr:wr + HE]

            # per-iteration index tile: s1|s2|xi|vg0|vg1|vg2
            ixt = wp.tile([128, 3072], I16, tag="idx")
            pab = "ab"[(it - 1) % 2]
            cab = "ab"[side]
            if it > 0:
                nc.sync.dma_start(ixt[:, 0:768], idx_d[f"s1{pab}"][:])
                nc.sync.dma_start(ixt[:, 768:1536], idx_d[f"s2{pab}"][:])
            nc.sync.dma_start(ixt[:, 1536:2304], idx_d["xi"][:])
            for s in range(DV):
                nc.sync.dma_start(ixt[:, 2304 + 256 * s:2304 + 256 * (s + 1)],
                                  idx_d[f"vg{s}{cab}"][:])
            ix_s1 = ixt[:, 0:768]
            ix_s2 = ixt[:, 768:1536]
            ix_xi = ixt[:, 1536:2304]

            if it > 0:
                pwin_off = T_LOCA if (it - 1) % 2 == 0 else T_FOR
                pwin = table[:, pwin_off:pwin_off + WIN]

            for c in range(N_ECH):
                cl = slice(c * ECH, (c + 1) * ECH)
                ic = slice(c * IC, (c + 1) * IC)
                G = wp.tile([128, 3 * ECH], F32, tag="G")
                g1, g2, g3 = G[:, 0:ECH], G[:, ECH:2 * ECH], G[:, 2 * ECH:3 * ECH]
                nc.gpsimd.ap_gather(g3, xwin, ix_xi[:, ic],
                                    channels=128, num_elems=N_VAR, d=1,
                                    num_idxs=ECH)
                if it == 0:
                    msg_ap = g3
                else:
                    nc.gpsimd.ap_gather(g1, pwin, ix_s1[:, ic],
                                        channels=128, num_elems=WIN, d=1,
                                        num_idxs=ECH)
                    nc.gpsimd.ap_gather(g2, pwin, ix_s2[:, ic],
                                        channels=128, num_elems=WIN, d=1,
                                        num_idxs=ECH)
                    pm = pp.tile([128, ECH], F32, tag="P1")
                    nc.vector.tensor_tensor(pm[:], g1, g2, op=ALU.add)
                    msg = pp.tile([128, ECH], F32, tag="P2")
                    nc.vector.tensor_tensor(msg[:], pm[:], g3, op=ALU.add)
                    msg_ap = msg[:]

                tt = pp.tile([128, ECH], F32, tag="P1")
                nc.scalar.activation(tt[:], msg_ap, ACT.Tanh, scale=0.5)
                sg = wp.tile([128, ECH], F32, tag="sgx")
                nc.scalar.activation(sg[:], tt[:], ACT.Sign)
                ab = pp.tile([128, ECH], F32, tag="P2")
                nc.scalar.activation(ab[:], tt[:], ACT.Abs)
                la = pp.tile([128, ECH], F32, tag="P3")
                nc.scalar.activation(la[:], ab[:], ACT.Ln, bias=sm_eps)

                la6 = la[:].rearrange("p (n k) -> p n k", k=DC)
                nc.vector.tensor_reduce(sm_csum, la6, axis=AX.X, op=ALU.add)

                sg6 = sg[:].rearrange("p (n k) -> p n k", k=DC)
                p3v = sm_p3.rearrange("p (n k) -> p n k", k=3)
                nc.vector.tensor_tensor(p3v, sg6[:, :, 0:3], sg6[:, :, 3:6],
                                        op=ALU.mult)
                nc.vector.tensor_tensor(sm_cp1, p3v[:, :, 0], p3v[:, :, 1],
                                        op=ALU.mult)
                nc.vector.tensor_tensor(sm_cp, sm_cp1, p3v[:, :, 2],
                                        op=ALU.mult)

                dd = pp.tile([128, ECH], F32, tag="P4")
                dd6 = dd[:].rearrange("p (n k) -> p n k", k=DC)
                csb = sm_csum.unsqueeze(2).broadcast_to([128, CCH, DC])
                nc.vector.tensor_tensor(dd6, csb, la6, op=ALU.subtract)

                t2 = pp.tile([128, ECH], F32, tag="P1")
                nc.scalar.activation(t2[:], dd[:], ACT.Tanh, scale=-0.5)
                t2c = pp.tile([128, ECH], F32, tag="P2")
                nc.vector.tensor_scalar_max(t2c[:], t2[:], TCLIP)

                se = pp.tile([128, ECH], F32, tag="P4")
                se6 = se[:].rearrange("p (n k) -> p n k", k=DC)
                cpb = sm_cp.unsqueeze(2).broadcast_to([128, CCH, DC])
                nc.vector.tensor_tensor(se6, sg6, cpb, op=ALU.mult)

                a5 = wp.tile([128, ECH], F32, tag="sgx")
                nc.scalar.activation(a5[:], t2c[:], ACT.Ln)

                # ext = (-a5) * se  ->  local write side
                nc.vector.scalar_tensor_tensor(
                    wloc[:, cl], a5[:], -1.0, se[:],
                    op0=ALU.mult, op1=ALU.mult)

            # cross-fill to the partner half's foreign region. Emitted after
            # ALL of this iteration's sibling gathers so they still read the
            # previous state's foreign values (the region is single-buffered).
            for c in range(N_ECH):
                cl = slice(c * ECH, (c + 1) * ECH)
                nc.sync.dma_start(
                    table[0:64, T_FOR:T_FOR + HE][:, cl],
                    wloc[64:128, cl])
                nc.sync.dma_start(
                    table[64:128, T_FOR:T_FOR + HE][:, cl],
                    wloc[0:64, cl])

            # out phase: reads current state window
            cwin_off = T_LOCA if side == 0 else T_FOR
            cwin = table[:, cwin_off:cwin_off + WIN]
            for vc in range(N_VCH):
                iv = slice(vc * IVC, (vc + 1) * IVC)
                geo = wp.tile([128, 3 * VCH], F32, tag="G")
                ge = [geo[:, s * VCH:(s + 1) * VCH] for s in range(DV)]
                for s in range(DV):
                    nc.gpsimd.ap_gather(
                        ge[s], cwin, ixt[:, 2304 + 256 * s:2304 + 256 * (s + 1)][:, iv],
                        channels=128, num_elems=WIN, d=1, num_idxs=VCH)
                vso = wp.tile([128, 2 * VCH], F32, tag="sgx")
                vs, ov = vso[:, 0:VCH], vso[:, VCH:2 * VCH]
                nc.vector.tensor_tensor(vs, ge[0], ge[1], op=ALU.add)
                nc.vector.tensor_tensor(ov, vs, ge[2], op=ALU.add)
                vl = slice(vc * VCH, (vc + 1) * VCH)
                nc.vector.tensor_tensor(
                    vs[0:64], ov[0:64],
                    table[0:64, T_X:T_X + HV][:, vl], op=ALU.add)
                nc.vector.tensor_tensor(
                    vs[64:128], ov[64:128],
                    table[64:128, T_X + HV:T_X + N_VAR][:, vl], op=ALU.add)
                nc.sync.dma_start(out_d[it, :, vc * VCH:(vc + 1) * VCH],
                                  vs[0:64])
                nc.sync.dma_start(
                    out_d[it, :, HV + vc * VCH:HV + (vc + 1) * VCH],
                    vs[64:128])

    nc.compile()
    return nc


def _numpy_fallback(llr, vi, ci):
    x = llr.T.astype(np.float32)
    scattered = x[vi]
    ext = np.zeros_like(scattered)
    outs = []
    for _ in range(N_ITER):
        vsum = np.zeros((N_VAR, x.shape[1]), np.float32)
        np.add.at(vsum, vi, ext)
        msg = (vsum[vi] - ext) + scattered
        t = np.tanh(msg * 0.5)
        la = np.log(np.abs(t) + EPS)
        sg = np.sign(t)
        cs = np.zeros((N_CHK, x.shape[1]), np.float32)
        np.add.at(cs, ci, la)
        cpr = np.ones((N_CHK, x.shape[1]), np.float32)
        np.multiply.at(cpr, ci, sg)
        loo = np.exp(cs[ci] - la) * (cpr[ci] * sg)
        loo = np.clip(loo, -float(_C), float(_C))
        ext = 2.0 * np.arctanh(loo)
        vs2 = np.zeros((N_VAR, x.shape[1]), np.float32)
        np.add.at(vs2, vi, ext)
        outs.append((vs2 + x).T)
    return np.stack(outs)


def kernel(llr, var_index, chk_index):
    llr = np.asarray(llr, np.float32)
    vi = np.asarray(var_index, np.int64).ravel()
    ci = np.asarray(chk_index, np.int64).ravel()
    assert llr.shape == (BATCH, N_VAR) and vi.shape == (E,) and ci.shape == (E,)

    regular = (np.array_equal(np.bincount(vi, minlength=N_VAR),
                              np.full(N_VAR, DV))
               and np.array_equal(np.bincount(ci, minlength=N_CHK),
                                  np.full(N_CHK, DC)))
    if not regular:
        return _numpy_fallback(llr, vi, ci).astype(np.float32)

    key = ("k", hash(vi.tobytes()), hash(ci.tobytes()))
    if key not in _CACHE:
        planes = _build_indices(vi, ci)
        nc = _build_bass()
        _CACHE[key] = (nc, planes)
    nc, planes = _CACHE[key]

    from concourse.bass_utils import run_bass_kernel_spmd
    in_maps = []
    for c in range(N_CORES):
        m = {nm: np.ascontiguousarray(v) for nm, v in planes.items()}
        m["llr"] = np.ascontiguousarray(llr[c * BC:(c + 1) * BC, :])
        in_maps.append(m)
    trace = os.environ.get("BASS_KERNEL_TRACE", "0") == "1"
    res = run_bass_kernel_spmd(nc, in_maps, list(range(N_CORES)), trace=trace)
    global _LAST_RESULTS
    _LAST_RESULTS = res
    out = np.concatenate([res.results[c]["out"] for c in range(N_CORES)],
                         axis=1)
    return np.ascontiguousarray(out, dtype=np.float32)


if __name__ == "__main__":
    sys.path.insert(0, os.path.dirname(os.path.abspath(__file__)))
    import reference
    inputs = {k: np.asarray(v) for k, v in reference.setup_inputs().items()}
    exp = np.asarray(reference.reference(**inputs))
    got = kernel(**inputs)
    err = np.max(np.abs(got - exp)) / (np.max(np.abs(exp)) + 1e-30)
    print("Relative error:", err)



# revision 3
# speedup vs baseline: 1.6145x; 1.6145x over previous
"""Trainium2 Bass kernel for NeuralSumProductModel (LDPC sum-product decoder).

Contract: kernel(**inputs) takes FULL inputs (llr [512,8192] f32,
var_index [24576] i32, chk_index [24576] i32) and returns the FULL
output [5, 512, 8192] f32, matching reference.reference().

Design v2 (per NeuronCore, batch sharded 512 -> 8 x 64):
  - partitions = (edge-half h, batch b): p = h*64 + b
  - edges in check-major order (6 per check); half h owns checks
    [h*2048,(h+1)*2048) = edge cols [h*12288,(h+1)*12288)
  - ap_gather costs ~27ns/index (linear), so the kernel is gather-bound;
    v2 minimizes gather indices via the identity
        msg_e = out_prev[var(e)] - ext_prev[e]
    where out = x + vsum is the per-iteration output. Per iteration:
    12288 msg indices + 3x4096 var-phase indices = 24576 (vs 49152 in
    the sibling-gather formulation).
  - table [128, 32768] f32: [ext 0:12288 | foreign 12288:24576 |
    out_prev 24576:32768]. ext written in place each iteration (old ext
    is only read positionally before the overwrite); foreign is
    cross-filled per check piece; out_prev rebuilt each var phase.
  - check phase: 4 pieces x 3072 cols, baseline numerics (tanh, sign,
    abs, ln, reduce6, phi involution) with big ACT/DVE ops; all compute
    hides under the gathers.
"""

import os
import sys

import numpy as np

for _p in ("/opt/trn_rl_repo", "/root/.axon_site/_ro/trn_rl_repo"):
    if os.path.isdir(_p) and _p not in sys.path:
        sys.path.insert(0, _p)

N_VAR, N_CHK, DV, DC = 8192, 4096, 3, 6
E = N_VAR * DV  # 24576
BATCH, N_ITER, N_CORES = 512, 5, 8
BC = BATCH // N_CORES           # 64 batch rows per core
HE = E // 2                     # 12288 edge cols per half
HV = N_VAR // 2                 # 4096 vars per half
NP_CHK = 4                      # check pieces per iteration
PW = HE // NP_CHK               # 3072 edge cols per piece
PC = PW // DC                   # 512 checks per piece
T_EXT, T_FOR, T_OP = 0, HE, 2 * HE
T_COLS = 2 * HE + N_VAR         # 32768
VWIN = 2 * HE                   # 24576-col var gather window

EPS = 1e-12
_C = np.float32(1.0) - np.float32(1e-7)
TCLIP = float(np.float32((np.float32(1.0) - _C) / (np.float32(1.0) + _C)))

_CACHE = {}
_LAST_RESULTS = None


def _wrap(stream):
    """Pack an unwrapped per-core index stream [8, n] -> wrapped [128, n//16].

    ap_gather unwraps core k's indices as unwrapped[s*16+p] = tile[16k+p, s].
    """
    st = np.asarray(stream, np.int16)
    ncore, n = st.shape
    assert n % 16 == 0
    out = np.zeros((16 * ncore, n // 16), np.int16)
    for k in range(ncore):
        out[16 * k:16 * (k + 1), :] = st[k].reshape(n // 16, 16).T
    return out


def _build_indices(vi, ci):
    """Host-side graph preprocessing. Returns dict of wrapped index planes."""
    order = np.argsort(ci, kind="stable")          # check-major edge list
    cm_var = vi[order].astype(np.int64)            # var of each cm edge
    pos_of_edge = np.empty(E, np.int64)
    pos_of_edge[order] = np.arange(E)
    edges_of_var = np.argsort(vi, kind="stable").reshape(N_VAR, DV)
    pos_var = pos_of_edge[edges_of_var]            # [N_VAR, 3] cm positions

    planes = {}
    # msg gather: out_prev window (8192 wide), idx = var id of each cm edge
    ixm = np.zeros((2, HE), np.int64)
    for H in (0, 1):
        ixm[H] = cm_var[H * HE:(H + 1) * HE]
    planes["ixm"] = _wrap(np.concatenate([np.repeat(ixm[0][None], 4, 0),
                                          np.repeat(ixm[1][None], 4, 0)]))

    # var gathers: window [ext | foreign] as seen from half H:
    # local-half position p -> p % HE ; other-half -> HE + p % HE
    for s in range(DV):
        vg = np.zeros((2, HV), np.int64)
        for H in (0, 1):
            vids = np.arange(H * HV, (H + 1) * HV)
            p = pos_var[vids, s]
            same = (p // HE) == H
            vg[H] = (p % HE) + HE * (~same)
        planes[f"vg{s}"] = _wrap(np.concatenate([np.repeat(vg[0][None], 4, 0),
                                                 np.repeat(vg[1][None], 4, 0)]))
    return planes


def _build_bass():
    import concourse.bass as bass
    import concourse.tile as tile
    from concourse import bacc, mybir
    from contextlib import ExitStack

    dt = mybir.dt
    F32, BF16, I16 = dt.float32, dt.bfloat16, dt.int16
    ALU = mybir.AluOpType
    ACT = mybir.ActivationFunctionType
    AX = mybir.AxisListType

    nc = bacc.Bacc("TRN2", target_bir_lowering=False, debug=False)

    llr_d = nc.dram_tensor("llr", [BC, N_VAR], F32, kind="ExternalInput").ap()
    idx_d = {}
    idx_d["ixm"] = nc.dram_tensor("ixm", [128, HE // 16], I16,
                                  kind="ExternalInput").ap()
    for s in range(DV):
        idx_d[f"vg{s}"] = nc.dram_tensor(f"vg{s}", [128, HV // 16], I16,
                                         kind="ExternalInput").ap()
    out_d = nc.dram_tensor("out", [N_ITER, BC, N_VAR], F32,
                           kind="ExternalOutput").ap()

    with tile.TileContext(nc) as tc, ExitStack() as ctx:
        big = ctx.enter_context(tc.tile_pool(name="big", bufs=1))

        table = big.tile([128, T_COLS], F32, tag="table")
        arena = big.tile([128, 12288], F32, tag="arena")
        sgt = big.tile([128, 2 * PW], BF16, tag="sgt")
        # smalls: csum f32 [512] + eps, plus bf16 sign products
        smf = big.tile([128, PC + 1], F32, tag="smf")
        smb = big.tile([128, 5 * PC], BF16, tag="smb")
        sm_csum = smf[:, 0:PC]
        sm_eps = smf[:, PC:PC + 1]
        sm_p3 = smb[:, 0:3 * PC]
        sm_cp1 = smb[:, 3 * PC:4 * PC]
        sm_cp = smb[:, 4 * PC:5 * PC]

        ixm_t = big.tile([128, HE // 16], I16, tag="ixm_t")
        vg_t = [big.tile([128, HV // 16], I16, tag=f"vg_t{s}",
                         name=f"vg_t{s}") for s in range(DV)]

        nc.vector.memset(sm_eps, EPS)
        # ext must start at 0 (iter 0 reads it positionally)
        nc.vector.memset(table[:, T_EXT:T_EXT + HE], 0.0)

        # one-time loads: indices + out_prev := x (duplicated across halves)
        nc.sync.dma_start(ixm_t[:], idx_d["ixm"][:])
        for s in range(DV):
            nc.sync.dma_start(vg_t[s][:], idx_d[f"vg{s}"][:])
        nc.sync.dma_start(table[0:64, T_OP:T_OP + N_VAR], llr_d[:, :])
        nc.sync.dma_start(table[64:128, T_OP:T_OP + N_VAR], llr_d[:, :])

        op_win = table[:, T_OP:T_OP + N_VAR]
        v_win = table[:, 0:VWIN]
        IC = PW // 16           # wrapped idx cols per check piece (192)
        IV = HV // 16           # wrapped idx cols per var gather (256)

        # arena layout (f32 cols): check pieces use two rotating slots
        TA = [arena[:, 0:PW], arena[:, PW:2 * PW]]
        TB = [arena[:, 2 * PW:3 * PW], arena[:, 3 * PW:4 * PW]]
        SG = [sgt[:, 0:PW], sgt[:, PW:2 * PW]]
        # var phase reuses the same arena space
        GA = arena[:, 0:HV]
        GB = arena[:, HV:2 * HV]
        XB = arena[:, 2 * HV:3 * HV]

        for it in range(N_ITER):
            # ---- check phase: 4 pieces of 3072 cols ----
            for p in range(NP_CHK):
                k = p % 2
                cl = slice(p * PW, (p + 1) * PW)
                ta, tb, sg = TA[k], TB[k], SG[k]

                nc.gpsimd.ap_gather(ta, op_win, ixm_t[:, p * IC:(p + 1) * IC],
                                    channels=128, num_elems=N_VAR, d=1,
                                    num_idxs=PW)
                # msg = gather(out_prev) - ext_prev
                nc.vector.tensor_tensor(tb, ta, table[:, cl], op=ALU.subtract)
                nc.scalar.activation(ta, tb, ACT.Tanh, scale=0.5)
                nc.scalar.activation(sg, ta, ACT.Sign)
                nc.scalar.activation(tb, ta, ACT.Abs)
                nc.scalar.activation(ta, tb, ACT.Ln, bias=sm_eps)

                la6 = ta.rearrange("p (n k) -> p n k", k=DC)
                nc.vector.tensor_reduce(sm_csum, la6, axis=AX.X, op=ALU.add)

                sg6 = sg.rearrange("p (n k) -> p n k", k=DC)
                p3v = sm_p3.rearrange("p (n k) -> p n k", k=3)
                nc.vector.tensor_tensor(p3v, sg6[:, :, 0:3], sg6[:, :, 3:6],
                                        op=ALU.mult)
                nc.vector.tensor_tensor(sm_cp1, p3v[:, :, 0], p3v[:, :, 1],
                                        op=ALU.mult)
                nc.vector.tensor_tensor(sm_cp, sm_cp1, p3v[:, :, 2],
                                        op=ALU.mult)

                dd6 = tb.rearrange("p (n k) -> p n k", k=DC)
                csb = sm_csum.unsqueeze(2).broadcast_to([128, PC, DC])
                nc.vector.tensor_tensor(dd6, csb, la6, op=ALU.subtract)

                nc.scalar.activation(ta, tb, ACT.Tanh, scale=-0.5)
                nc.vector.tensor_scalar_max(tb, ta, TCLIP)
                nc.scalar.activation(ta, tb, ACT.Ln)

                # se = sg * cprod -> tb ; ext = (-a5) * se
                se6 = tb.rearrange("p (n k) -> p n k", k=DC)
                cpb = sm_cp.unsqueeze(2).broadcast_to([128, PC, DC])
                nc.vector.tensor_tensor(se6, sg6, cpb, op=ALU.mult)
                nc.vector.scalar_tensor_tensor(
                    table[:, cl], ta, -1.0, tb, op0=ALU.mult, op1=ALU.mult)

                # cross-fill this piece's new ext to the partner half
                nc.sync.dma_start(table[0:64, T_FOR + p * PW:T_FOR + (p + 1) * PW],
                                  table[64:128, cl])
                nc.sync.dma_start(table[64:128, T_FOR + p * PW:T_FOR + (p + 1) * PW],
                                  table[0:64, cl])

            # x halves for the var phase (XB overlaps piece buffers, so load
            # only after the check pieces; overlaps with the var gathers)
            nc.sync.dma_start(XB[0:64], llr_d[:, 0:HV])
            nc.sync.dma_start(XB[64:128], llr_d[:, HV:N_VAR])

            # ---- var phase: vsum via 3 gathers, out = x + vsum ----
            opl_a = table[0:64, T_OP:T_OP + HV]
            opl_b = table[64:128, T_OP + HV:T_OP + N_VAR]
            nc.gpsimd.ap_gather(GA, v_win, vg_t[0][:],
                                channels=128, num_elems=VWIN, d=1, num_idxs=HV)
            nc.gpsimd.ap_gather(GB, v_win, vg_t[1][:],
                                channels=128, num_elems=VWIN, d=1, num_idxs=HV)
            nc.vector.tensor_tensor(opl_a, GA[0:64], GB[0:64], op=ALU.add)
            nc.vector.tensor_tensor(opl_b, GA[64:128], GB[64:128], op=ALU.add)
            nc.gpsimd.ap_gather(GB, v_win, vg_t[2][:],
                                channels=128, num_elems=VWIN, d=1, num_idxs=HV)
            nc.vector.tensor_tensor(GA[0:64], opl_a, GB[0:64], op=ALU.add)
            nc.vector.tensor_tensor(GA[64:128], opl_b, GB[64:128], op=ALU.add)
            nc.vector.tensor_tensor(opl_a, GA[0:64], XB[0:64], op=ALU.add)
            nc.vector.tensor_tensor(opl_b, GA[64:128], XB[64:128], op=ALU.add)

            # write this iteration's output rows
            nc.sync.dma_start(out_d[it, :, 0:HV], opl_a)
            nc.sync.dma_start(out_d[it, :, HV:N_VAR], opl_b)
            # cross-fill out_prev's foreign half for the next iteration
            if it + 1 < N_ITER:
                nc.sync.dma_start(table[0:64, T_OP + HV:T_OP + N_VAR],
                                  table[64:128, T_OP + HV:T_OP + N_VAR])
                nc.sync.dma_start(table[64:128, T_OP:T_OP + HV],
                                  table[0:64, T_OP:T_OP + HV])

    nc.compile()
    return nc


def _numpy_fallback(llr, vi, ci):
    x = llr.T.astype(np.float32)
    scattered = x[vi]
    ext = np.zeros_like(scattered)
    outs = []
    for _ in range(N_ITER):
        vsum = np.zeros((N_VAR, x.shape[1]), np.float32)
        np.add.at(vsum, vi, ext)
        msg = (vsum[vi] - ext) + scattered
        t = np.tanh(msg * 0.5)
        la = np.log(np.abs(t) + EPS)
        sg = np.sign(t)
        cs = np.zeros((N_CHK, x.shape[1]), np.float32)
        np.add.at(cs, ci, la)
        cpr = np.ones((N_CHK, x.shape[1]), np.float32)
        np.multiply.at(cpr, ci, sg)
        loo = np.exp(cs[ci] - la) * (cpr[ci] * sg)
        loo = np.clip(loo, -float(_C), float(_C))
        ext = 2.0 * np.arctanh(loo)
        vs2 = np.zeros((N_VAR, x.shape[1]), np.float32)
        np.add.at(vs2, vi, ext)
        outs.append((vs2 + x).T)
    return np.stack(outs)


def kernel(llr, var_index, chk_index):
    llr = np.asarray(llr, np.float32)
    vi = np.asarray(var_index, np.int64).ravel()
    ci = np.asarray(chk_index, np.int64).ravel()
    assert llr.shape == (BATCH, N_VAR) and vi.shape == (E,) and ci.shape == (E,)

    regular = (np.array_equal(np.bincount(vi, minlength=N_VAR),
                              np.full(N_VAR, DV))
               and np.array_equal(np.bincount(ci, minlength=N_CHK),
                                  np.full(N_CHK, DC)))
    if not regular:
        return _numpy_fallback(llr, vi, ci).astype(np.float32)

    key = ("k2", hash(vi.tobytes()), hash(ci.tobytes()))
    if key not in _CACHE:
        planes = _build_indices(vi, ci)
        nc = _build_bass()
        _CACHE[key] = (nc, planes)
    nc, planes = _CACHE[key]

    from concourse.bass_utils import run_bass_kernel_spmd
    in_maps = []
    for c in range(N_CORES):
        m = {nm: np.ascontiguousarray(v) for nm, v in planes.items()}
        m["llr"] = np.ascontiguousarray(llr[c * BC:(c + 1) * BC, :])
        in_maps.append(m)
    trace = os.environ.get("BASS_KERNEL_TRACE", "0") == "1"
    res = run_bass_kernel_spmd(nc, in_maps, list(range(N_CORES)), trace=trace)
    global _LAST_RESULTS
    _LAST_RESULTS = res
    out = np.concatenate([res.results[c]["out"] for c in range(N_CORES)],
                         axis=1)
    return np.ascontiguousarray(out, dtype=np.float32)


if __name__ == "__main__":
    sys.path.insert(0, os.path.dirname(os.path.abspath(__file__)))
    import reference
    inputs = {k: np.asarray(v) for k, v in reference.setup_inputs().items()}
    exp = np.asarray(reference.reference(**inputs))
    got = kernel(**inputs)
    err = np.max(np.abs(got - exp)) / (np.max(np.abs(exp)) + 1e-30)
    print("Relative error:", err)


# revision 18
# speedup vs baseline: 2.1909x; 1.3570x over previous
"""Trainium2 Bass kernel for NeuralSumProductModel (LDPC sum-product decoder).

Contract: kernel(**inputs) takes FULL inputs (llr [512,8192] f32,
var_index [24576] i32, chk_index [24576] i32) and returns the FULL
output [5, 512, 8192] f32, matching reference.reference().

Design v3 (per NeuronCore, batch sharded 512 -> 8 x 64): batch-on-free
layout + dma_gather. ap_gather moves 4B per index (~27ns/idx); dma_gather
moves a 256B row (64 f32 = one batch row) per descriptor at ~0.34ns/desc
generation and DMA-bus execution, so all permutation traffic rides DMA.

  - SBUF check layout: partition p owns checks {ct*128+p : ct in [0,32)},
    cols [ct][e][b] (e in [0,6) edge slot, b in [0,64) batch).
  - msg_e = out_prev[var(e)] - ext_prev[e]: out rows live in DRAM OUTR
    [8192, 64]; dma_gather pulls row var(e) for each edge slot.
  - check phase: 4 pieces x 8 check-tiles (3072 cols), baseline numerics
    (tanh, sign, abs, ln, strided reduce6, phi involution, sign via
    reduce-mult). New ext written to SBUF EXTSB (positional reuse next
    iteration) and streamed to DRAM EXTR [24576, 64] rows r=p*192+ct*6+e.
  - var phase: dma_gather pulls ext rows at each var's 3 edge positions
    -> VG [p, vt, s, b]; vsum = reduce over s; out = vsum + x; written
    back to OUTR (v-major rows) and to out_d (batch-major) from a
    transposed copy.
"""

import os
import sys

import numpy as np

for _p in ("/opt/trn_rl_repo", "/root/.axon_site/_ro/trn_rl_repo"):
    if os.path.isdir(_p) and _p not in sys.path:
        sys.path.insert(0, _p)

N_VAR, N_CHK, DV, DC = 8192, 4096, 3, 6
E = N_VAR * DV  # 24576
BATCH, N_ITER, N_CORES = 512, 5, 8
BC = BATCH // N_CORES           # 64 batch rows per core
NCT = N_CHK // 128              # 32 check tiles
NVT = N_VAR // 128              # 64 var blocks per partition (v = p*64 + vt)
NP_CHK = 4                      # check pieces per iteration
CTP = NCT // NP_CHK             # 8 check tiles per piece
PW = CTP * DC * BC              # 3072 cols per piece
W = NCT * DC * BC               # 12288 cols total (ext per partition)

GCH = 1024                      # dma_gather rows per call (SWDGE ring cap)

EPS = 1e-12
_C = np.float32(1.0) - np.float32(1e-7)
TCLIP = float(np.float32((np.float32(1.0) - _C) / (np.float32(1.0) + _C)))

_CACHE = {}
_LAST_RESULTS = None


def _wrap(stream):
    """Pack an index stream [n] -> wrapped [128, n//16], replicated across
    the 8 gpsimd cores (dma_gather uses one shared stream)."""
    st = np.asarray(stream, np.int16)
    n = st.shape[0]
    assert n % 16 == 0
    core = st.reshape(n // 16, 16).T     # [16, n//16]
    return np.tile(core, (8, 1))


def _build_indices(vi, ci):
    """Host-side graph preprocessing. Returns dict of wrapped index planes."""
    order = np.argsort(ci, kind="stable")          # check-major edge list
    cm_var = vi[order].astype(np.int64)            # var of each cm edge
    pos_of_edge = np.empty(E, np.int64)
    pos_of_edge[order] = np.arange(E)
    edges_of_var = np.argsort(vi, kind="stable").reshape(N_VAR, DV)
    pos_var = pos_of_edge[edges_of_var]            # [N_VAR, 3] cm positions

    # msg gather: OUTR row = var id of edge slot (c = ct*128 + p, e);
    # output slot j = (ct_loc*6 + e)*128 + p within each piece.
    ixm = np.zeros(E, np.int64)
    for pc in range(NP_CHK):
        for jl in range(CTP * DC):
            ctl, e = jl // DC, jl % DC
            c = (pc * CTP + ctl) * 128 + np.arange(128)
            ixm[pc * CTP * DC * 128 + jl * 128:
                pc * CTP * DC * 128 + (jl + 1) * 128] = cm_var[c * DC + e]
    planes = {"ixm": _wrap(ixm)}

    # var gather: EXTR row of var v's s-th edge; v = p*64 + vt,
    # output slot j = (vt*3 + s)*128 + p.
    vidx = np.zeros(DV * N_VAR, np.int64)
    for vt in range(NVT):
        for s in range(DV):
            p = np.arange(128)
            v = p * NVT + vt
            j = pos_var[v, s]                      # cm position
            c, e = j // DC, j % DC
            r = (c % 128) * (NCT * DC) + (c // 128) * DC + e
            vidx[(vt * DV + s) * 128 + p] = r
    planes["vidx"] = _wrap(vidx)
    return planes


def _build_bass():
    import concourse.bass as bass
    import concourse.tile as tile
    from concourse import bacc, mybir
    from contextlib import ExitStack

    dt = mybir.dt
    F32, BF16, I16 = dt.float32, dt.bfloat16, dt.int16
    ALU = mybir.AluOpType
    ACT = mybir.ActivationFunctionType
    AX = mybir.AxisListType

    nc = bacc.Bacc("TRN2", target_bir_lowering=False, debug=False)

    llr_d = nc.dram_tensor("llr", [BC, N_VAR], F32, kind="ExternalInput").ap()
    ixm_d = nc.dram_tensor("ixm", [128, E // 16], I16,
                           kind="ExternalInput").ap()
    vidx_d = nc.dram_tensor("vidx", [128, DV * N_VAR // 16], I16,
                            kind="ExternalInput").ap()
    out_d = nc.dram_tensor("out", [N_ITER, BC, N_VAR], F32,
                           kind="ExternalOutput").ap()
    outr = nc.dram_tensor("outr", [N_VAR, BC], F32, kind="Internal").ap()
    extr = nc.dram_tensor("extr", [E, BC], F32, kind="Internal").ap()

    outr_flat = outr[:, :].rearrange("(p n) k -> p (n k)", p=128)
    extr_flat = extr[:, :].rearrange("(p n) k -> p (n k)", p=128)
    llr_bv = llr_d[:, :].rearrange("b (p vt) -> p b vt", p=128)

    with tile.TileContext(nc) as tc, ExitStack() as ctx:
        big = ctx.enter_context(tc.tile_pool(name="big", bufs=1))

        arena = big.tile([128, W], F32, tag="arena")         # 48KB
        extsb = big.tile([128, W], F32, tag="extsb")         # 48KB
        sgt = big.tile([128, 2 * PW], BF16, tag="sgt")       # 12KB
        out_v = big.tile([128, N_VAR // 2], F32, tag="out_v")   # 16KB
        out_b = big.tile([128, N_VAR // 2], F32, tag="out_b")   # 16KB
        xsb = big.tile([128, N_VAR // 2], F32, tag="xsb")    # 16KB (b-major)
        csum = big.tile([128, 2 * CTP * BC], F32, tag="csum")
        cpt = big.tile([128, 2 * CTP * BC], BF16, tag="cpt")
        epst = big.tile([128, 1], F32, tag="epst")
        ixm_t = big.tile([128, E // 16], I16, tag="ixm_t")
        vidx_t = big.tile([128, DV * N_VAR // 16], I16, tag="vidx_t")

        TA = [arena[:, 0:PW], arena[:, PW:2 * PW]]
        TB = [arena[:, 2 * PW:3 * PW], arena[:, 3 * PW:4 * PW]]
        SG = [sgt[:, 0:PW], sgt[:, PW:2 * PW]]
        CS = [csum[:, 0:CTP * BC], csum[:, CTP * BC:2 * CTP * BC]]
        CP = [cpt[:, 0:CTP * BC], cpt[:, CTP * BC:2 * CTP * BC]]

        nc.vector.memset(epst[:], EPS)
        nc.vector.memset(extsb[:], 0.0)
        nc.sync.dma_start(ixm_t[:], ixm_d[:])
        nc.sync.dma_start(vidx_t[:], vidx_d[:])
        nc.sync.dma_start(xsb[:].rearrange("p (b vt) -> p b vt", b=BC),
                          llr_bv)

        # OUTR := x rows (v-major): out_v <- transpose copy of xsb, then DMA
        xsb_vv = xsb[:].rearrange("p (b vt) -> p vt b", b=BC)
        ovv = out_v[:].rearrange("p (vt b) -> p vt b", vt=NVT)
        nc.vector.tensor_scalar_add(ovv, xsb_vv, 0.0)
        nc.sync.dma_start(outr_flat, out_v[:])

        for it in range(N_ITER):
            for pc in range(NP_CHK):
                k = pc % 2
                cl = slice(pc * PW, (pc + 1) * PW)
                ta, tb, sg, cs, cp = TA[k], TB[k], SG[k], CS[k], CP[k]

                nip = CTP * DC * 128        # 6144 gathered rows per piece
                # SWDGE ring caps a single dma_gather at ~1024 descriptors
                # (larger calls wedge the device); chunk the gather.
                for g0 in range(0, nip, GCH):
                    dsl = ta[:, (g0 // 128) * BC:((g0 + GCH) // 128) * BC]
                    nc.gpsimd.dma_gather(
                        dsl.rearrange("p (n k) -> p n k", k=BC), outr[:, :],
                        ixm_t[:, (pc * nip + g0) // 16:
                              (pc * nip + g0 + GCH) // 16],
                        num_idxs=GCH, num_idxs_reg=GCH, elem_size=BC)

                # msg = gather(out_prev) - ext_prev
                nc.vector.tensor_tensor(tb, ta, extsb[:, cl],
                                        op=ALU.subtract)
                nc.scalar.activation(ta, tb, ACT.Tanh, scale=0.5)
                nc.scalar.activation(sg, ta, ACT.Sign)
                nc.scalar.activation(tb, ta, ACT.Abs)
                nc.scalar.activation(ta, tb, ACT.Ln, bias=epst[:])

                la6 = ta.rearrange("p (ct e b) -> p ct b e", ct=CTP, e=DC)
                cs6 = cs.rearrange("p (ct b) -> p ct b", ct=CTP)
                nc.vector.tensor_reduce(cs6, la6, axis=AX.X, op=ALU.add)
                sg6 = sg.rearrange("p (ct e b) -> p ct b e", ct=CTP, e=DC)
                cp6 = cp.rearrange("p (ct b) -> p ct b", ct=CTP)
                nc.vector.tensor_reduce(cp6, sg6, axis=AX.X, op=ALU.mult)

                dd6 = tb.rearrange("p (ct e b) -> p ct b e", ct=CTP, e=DC)
                csb = cs6.unsqueeze(3).broadcast_to([128, CTP, BC, DC])
                nc.vector.tensor_tensor(dd6, csb, la6, op=ALU.subtract)

                nc.scalar.activation(ta, tb, ACT.Tanh, scale=-0.5)
                nc.vector.tensor_scalar_max(tb, ta, TCLIP)
                nc.scalar.activation(ta, tb, ACT.Ln)

                se6 = tb.rearrange("p (ct e b) -> p ct b e", ct=CTP, e=DC)
                cpb = cp6.unsqueeze(3).broadcast_to([128, CTP, BC, DC])
                nc.vector.tensor_tensor(se6, sg6, cpb, op=ALU.mult)
                nc.vector.scalar_tensor_tensor(
                    extsb[:, cl], ta, -1.0, tb, op0=ALU.mult, op1=ALU.mult)

                nc.sync.dma_start(extr_flat[:, cl], extsb[:, cl])

            # ---- var phase ----
            for g0 in range(0, DV * N_VAR, GCH):
                dsl = arena[:, (g0 // 128) * BC:((g0 + GCH) // 128) * BC]
                nc.gpsimd.dma_gather(
                    dsl.rearrange("p (n k) -> p n k", k=BC), extr[:, :],
                    vidx_t[:, g0 // 16:(g0 + GCH) // 16],
                    num_idxs=GCH, num_idxs_reg=GCH, elem_size=BC)
            vg4 = arena[:].rearrange("p (vt s b) -> p vt b s", vt=NVT, s=DV)
            obv = out_b[:].rearrange("p (vt b) -> p vt b", vt=NVT)
            nc.vector.tensor_reduce(obv, vg4, axis=AX.X, op=ALU.add)
            nc.vector.tensor_tensor(ovv, obv, xsb_vv, op=ALU.add)
            if it + 1 < N_ITER:
                nc.sync.dma_start(outr_flat, out_v[:])
            # transpose copy to batch-major and write the iteration output
            obb = out_b[:].rearrange("p (b vt) -> p vt b", b=BC)
            nc.vector.tensor_scalar_add(obb, ovv, 0.0)
            od_bv = out_d[it].rearrange("b (p vt) -> p b vt", p=128)
            nc.sync.dma_start(od_bv,
                              out_b[:].rearrange("p (b vt) -> p b vt", b=BC))

    nc.compile()
    return nc


def _numpy_fallback(llr, vi, ci):
    x = llr.T.astype(np.float32)
    scattered = x[vi]
    ext = np.zeros_like(scattered)
    outs = []
    for _ in range(N_ITER):
        vsum = np.zeros((N_VAR, x.shape[1]), np.float32)
        np.add.at(vsum, vi, ext)
        msg = (vsum[vi] - ext) + scattered
        t = np.tanh(msg * 0.5)
        la = np.log(np.abs(t) + EPS)
        sg = np.sign(t)
        cs = np.zeros((N_CHK, x.shape[1]), np.float32)
        np.add.at(cs, ci, la)
        cpr = np.ones((N_CHK, x.shape[1]), np.float32)
        np.multiply.at(cpr, ci, sg)
        loo = np.exp(cs[ci] - la) * (cpr[ci] * sg)
        loo = np.clip(loo, -float(_C), float(_C))
        ext = 2.0 * np.arctanh(loo)
        vs2 = np.zeros((N_VAR, x.shape[1]), np.float32)
        np.add.at(vs2, vi, ext)
        outs.append((vs2 + x).T)
    return np.stack(outs)


def kernel(llr, var_index, chk_index):
    llr = np.asarray(llr, np.float32)
    vi = np.asarray(var_index, np.int64).ravel()
    ci = np.asarray(chk_index, np.int64).ravel()
    assert llr.shape == (BATCH, N_VAR) and vi.shape == (E,) and ci.shape == (E,)

    regular = (np.array_equal(np.bincount(vi, minlength=N_VAR),
                              np.full(N_VAR, DV))
               and np.array_equal(np.bincount(ci, minlength=N_CHK),
                                  np.full(N_CHK, DC)))
    if not regular:
        return _numpy_fallback(llr, vi, ci).astype(np.float32)

    key = ("k3", hash(vi.tobytes()), hash(ci.tobytes()))
    if key not in _CACHE:
        planes = _build_indices(vi, ci)
        nc = _build_bass()
        _CACHE[key] = (nc, planes)
    nc, planes = _CACHE[key]

    from concourse.bass_utils import run_bass_kernel_spmd
    in_maps = []
    for c in range(N_CORES):
        m = {nm: np.ascontiguousarray(v) for nm, v in planes.items()}
        m["llr"] = np.ascontiguousarray(llr[c * BC:(c + 1) * BC, :])
        in_maps.append(m)
    trace = os.environ.get("BASS_KERNEL_TRACE", "0") == "1"
    res = run_bass_kernel_spmd(nc, in_maps, list(range(N_CORES)), trace=trace)
    global _LAST_RESULTS
    _LAST_RESULTS = res
    out = np.concatenate([res.results[c]["out"] for c in range(N_CORES)],
                         axis=1)
    return np.ascontiguousarray(out, dtype=np.float32)


if __name__ == "__main__":
    sys.path.insert(0, os.path.dirname(os.path.abspath(__file__)))
    import reference
    inputs = {k: np.asarray(v) for k, v in reference.setup_inputs().items()}
    exp = np.asarray(reference.reference(**inputs))
    got = kernel(**inputs)
    err = np.max(np.abs(got - exp)) / (np.max(np.abs(exp)) + 1e-30)
    print("Relative error:", err)


# revision 21
# speedup vs baseline: 4.1620x; 1.8997x over previous
"""Trainium2 Bass kernel for NeuralSumProductModel (LDPC sum-product decoder).

Contract: kernel(**inputs) takes FULL inputs (llr [512,8192] f32,
var_index [24576] i32, chk_index [24576] i32) and returns the FULL
output [5, 512, 8192] f32, matching reference.reference().

Design v3 (per NeuronCore, batch sharded 512 -> 8 x 64): batch-on-free
layout + dma_gather. ap_gather moves 4B per index (~27ns/idx); dma_gather
moves a 256B row (64 f32 = one batch row) per descriptor at ~0.34ns/desc
generation and DMA-bus execution, so all permutation traffic rides DMA.

  - SBUF check layout: partition p owns checks {ct*128+p : ct in [0,32)},
    cols [ct][e][b] (e in [0,6) edge slot, b in [0,64) batch).
  - msg_e = out_prev[var(e)] - ext_prev[e]: out rows live in DRAM OUTR
    [8192, 64]; dma_gather pulls row var(e) for each edge slot.
  - check phase: 4 pieces x 8 check-tiles (3072 cols), baseline numerics
    (tanh, sign, abs, ln, strided reduce6, phi involution, sign via
    reduce-mult). New ext written to SBUF EXTSB (positional reuse next
    iteration) and streamed to DRAM EXTR [24576, 64] rows r=p*192+ct*6+e.
  - var phase: dma_gather pulls ext rows at each var's 3 edge positions
    -> VG [p, vt, s, b]; vsum = reduce over s; out = vsum + x; written
    back to OUTR (v-major rows) and to out_d (batch-major) from a
    transposed copy.
"""

import os
import sys

import numpy as np

for _p in ("/opt/trn_rl_repo", "/root/.axon_site/_ro/trn_rl_repo"):
    if os.path.isdir(_p) and _p not in sys.path:
        sys.path.insert(0, _p)

N_VAR, N_CHK, DV, DC = 8192, 4096, 3, 6
E = N_VAR * DV  # 24576
BATCH, N_ITER, N_CORES = 512, 5, 8
BC = BATCH // N_CORES           # 64 batch rows per core
NCT = N_CHK // 128              # 32 check tiles
NVT = N_VAR // 128              # 64 var blocks per partition (v = p*64 + vt)
NP_CHK = 4                      # check pieces per iteration
CTP = NCT // NP_CHK             # 8 check tiles per piece
PW = CTP * DC * BC              # 3072 cols per piece
W = NCT * DC * BC               # 12288 cols total (ext per partition)

GCH = 1024                      # dma_gather rows per call (SWDGE ring cap)

EPS = 1e-12
_C = np.float32(1.0) - np.float32(1e-7)
TCLIP = float(np.float32((np.float32(1.0) - _C) / (np.float32(1.0) + _C)))

_CACHE = {}
_LAST_RESULTS = None


def _wrap(stream):
    """Pack an index stream [n] -> wrapped [128, n//16], replicated across
    the 8 gpsimd cores (dma_gather uses one shared stream)."""
    st = np.asarray(stream, np.int16)
    n = st.shape[0]
    assert n % 16 == 0
    core = st.reshape(n // 16, 16).T     # [16, n//16]
    return np.tile(core, (8, 1))


def _build_indices(vi, ci):
    """Host-side graph preprocessing. Returns dict of wrapped index planes."""
    order = np.argsort(ci, kind="stable")          # check-major edge list
    cm_var = vi[order].astype(np.int64)            # var of each cm edge
    pos_of_edge = np.empty(E, np.int64)
    pos_of_edge[order] = np.arange(E)
    edges_of_var = np.argsort(vi, kind="stable").reshape(N_VAR, DV)
    pos_var = pos_of_edge[edges_of_var]            # [N_VAR, 3] cm positions

    # msg gather: OUTR row = var id of edge slot (c = ct*128 + p, e);
    # output slot j = (ct_loc*6 + e)*128 + p within each piece.
    ixm = np.zeros(E, np.int64)
    for pc in range(NP_CHK):
        for jl in range(CTP * DC):
            ctl, e = jl // DC, jl % DC
            c = (pc * CTP + ctl) * 128 + np.arange(128)
            ixm[pc * CTP * DC * 128 + jl * 128:
                pc * CTP * DC * 128 + (jl + 1) * 128] = cm_var[c * DC + e]
    planes = {"ixm": _wrap(ixm)}

    # var gather: EXTR row of var v's s-th edge; v = p*64 + vt,
    # output slot j = (vt*3 + s)*128 + p.
    vidx = np.zeros(DV * N_VAR, np.int64)
    for vt in range(NVT):
        for s in range(DV):
            p = np.arange(128)
            v = p * NVT + vt
            j = pos_var[v, s]                      # cm position
            c, e = j // DC, j % DC
            r = (c % 128) * (NCT * DC) + (c // 128) * DC + e
            vidx[(vt * DV + s) * 128 + p] = r
    planes["vidx"] = _wrap(vidx)
    return planes


def _build_bass():
    import concourse.bass as bass
    import concourse.tile as tile
    from concourse import bacc, mybir
    from contextlib import ExitStack

    dt = mybir.dt
    F32, BF16, I16 = dt.float32, dt.bfloat16, dt.int16
    ALU = mybir.AluOpType
    ACT = mybir.ActivationFunctionType
    AX = mybir.AxisListType

    # 4 SWDGE queues: each dma_gather call's await_space blocks only on its
    # own ring, so desc-gen and DMA execution overlap across queues.
    nc = bacc.Bacc("TRN2", target_bir_lowering=False, debug=False,
                   num_swdge_queues=4)

    llr_d = nc.dram_tensor("llr", [BC, N_VAR], F32, kind="ExternalInput").ap()
    ixm_d = nc.dram_tensor("ixm", [128, E // 16], I16,
                           kind="ExternalInput").ap()
    vidx_d = nc.dram_tensor("vidx", [128, DV * N_VAR // 16], I16,
                            kind="ExternalInput").ap()
    out_d = nc.dram_tensor("out", [N_ITER, BC, N_VAR], F32,
                           kind="ExternalOutput").ap()
    outr = nc.dram_tensor("outr", [N_VAR, BC], F32, kind="Internal").ap()
    extr = nc.dram_tensor("extr", [E, BC], F32, kind="Internal").ap()

    outr_flat = outr[:, :].rearrange("(p n) k -> p (n k)", p=128)
    extr_flat = extr[:, :].rearrange("(p n) k -> p (n k)", p=128)
    llr_bv = llr_d[:, :].rearrange("b (p vt) -> p b vt", p=128)

    with tile.TileContext(nc) as tc, ExitStack() as ctx:
        big = ctx.enter_context(tc.tile_pool(name="big", bufs=1))

        arena = big.tile([128, W], F32, tag="arena")         # 48KB
        extsb = big.tile([128, W], F32, tag="extsb")         # 48KB
        sgt = big.tile([128, 2 * PW], BF16, tag="sgt")       # 12KB
        out_v = big.tile([128, N_VAR // 2], F32, tag="out_v")   # 16KB
        out_b = big.tile([128, N_VAR // 2], F32, tag="out_b")   # 16KB
        xsb = big.tile([128, N_VAR // 2], F32, tag="xsb")    # 16KB (b-major)
        csum = big.tile([128, 2 * CTP * BC], F32, tag="csum")
        cpt = big.tile([128, 2 * CTP * BC], BF16, tag="cpt")
        epst = big.tile([128, 1], F32, tag="epst")
        ixm_t = big.tile([128, E // 16], I16, tag="ixm_t")
        vidx_t = big.tile([128, DV * N_VAR // 16], I16, tag="vidx_t")

        TA = [arena[:, 0:PW], arena[:, PW:2 * PW]]
        TB = [arena[:, 2 * PW:3 * PW], arena[:, 3 * PW:4 * PW]]
        SG = [sgt[:, 0:PW], sgt[:, PW:2 * PW]]
        CS = [csum[:, 0:CTP * BC], csum[:, CTP * BC:2 * CTP * BC]]
        CP = [cpt[:, 0:CTP * BC], cpt[:, CTP * BC:2 * CTP * BC]]

        nc.vector.memset(epst[:], EPS)
        nc.vector.memset(extsb[:], 0.0)
        nc.sync.dma_start(ixm_t[:], ixm_d[:])
        nc.sync.dma_start(vidx_t[:], vidx_d[:])
        nc.sync.dma_start(xsb[:].rearrange("p (b vt) -> p b vt", b=BC),
                          llr_bv)

        # OUTR := x rows (v-major): out_v <- transpose copy of xsb, then DMA
        xsb_vv = xsb[:].rearrange("p (b vt) -> p vt b", b=BC)
        ovv = out_v[:].rearrange("p (vt b) -> p vt b", vt=NVT)
        nc.vector.tensor_scalar_add(ovv, xsb_vv, 0.0)
        nc.sync.dma_start(outr_flat, out_v[:])

        for it in range(N_ITER):
            for pc in range(NP_CHK):
                k = pc % 2
                cl = slice(pc * PW, (pc + 1) * PW)
                ta, tb, sg, cs, cp = TA[k], TB[k], SG[k], CS[k], CP[k]

                nip = CTP * DC * 128        # 6144 gathered rows per piece
                # SWDGE ring caps a single dma_gather at ~1024 descriptors
                # (larger calls wedge the device); chunk the gather.
                for g0 in range(0, nip, GCH):
                    dsl = ta[:, (g0 // 128) * BC:((g0 + GCH) // 128) * BC]
                    nc.gpsimd.dma_gather(
                        dsl.rearrange("p (n k) -> p n k", k=BC), outr[:, :],
                        ixm_t[:, (pc * nip + g0) // 16:
                              (pc * nip + g0 + GCH) // 16],
                        num_idxs=GCH, num_idxs_reg=GCH, elem_size=BC,
                        queue_num=(g0 // GCH) % 4)

                # msg = gather(out_prev) - ext_prev
                nc.vector.tensor_tensor(tb, ta, extsb[:, cl],
                                        op=ALU.subtract)
                nc.scalar.activation(ta, tb, ACT.Tanh, scale=0.5)
                nc.scalar.activation(sg, ta, ACT.Sign)
                nc.scalar.activation(tb, ta, ACT.Abs)
                nc.scalar.activation(ta, tb, ACT.Ln, bias=epst[:])

                la6 = ta.rearrange("p (ct e b) -> p ct b e", ct=CTP, e=DC)
                cs6 = cs.rearrange("p (ct b) -> p ct b", ct=CTP)
                nc.vector.tensor_reduce(cs6, la6, axis=AX.X, op=ALU.add)
                sg6 = sg.rearrange("p (ct e b) -> p ct b e", ct=CTP, e=DC)
                cp6 = cp.rearrange("p (ct b) -> p ct b", ct=CTP)
                nc.vector.tensor_reduce(cp6, sg6, axis=AX.X, op=ALU.mult)

                dd6 = tb.rearrange("p (ct e b) -> p ct b e", ct=CTP, e=DC)
                csb = cs6.unsqueeze(3).broadcast_to([128, CTP, BC, DC])
                nc.vector.tensor_tensor(dd6, csb, la6, op=ALU.subtract)

                nc.scalar.activation(ta, tb, ACT.Tanh, scale=-0.5)
                nc.vector.tensor_scalar_max(tb, ta, TCLIP)
                nc.scalar.activation(ta, tb, ACT.Ln)

                se6 = tb.rearrange("p (ct e b) -> p ct b e", ct=CTP, e=DC)
                cpb = cp6.unsqueeze(3).broadcast_to([128, CTP, BC, DC])
                nc.vector.tensor_tensor(se6, sg6, cpb, op=ALU.mult)
                nc.vector.scalar_tensor_tensor(
                    extsb[:, cl], ta, -1.0, tb, op0=ALU.mult, op1=ALU.mult)

                nc.sync.dma_start(extr_flat[:, cl], extsb[:, cl])

            # ---- var phase ----
            for g0 in range(0, DV * N_VAR, GCH):
                dsl = arena[:, (g0 // 128) * BC:((g0 + GCH) // 128) * BC]
                nc.gpsimd.dma_gather(
                    dsl.rearrange("p (n k) -> p n k", k=BC), extr[:, :],
                    vidx_t[:, g0 // 16:(g0 + GCH) // 16],
                    num_idxs=GCH, num_idxs_reg=GCH, elem_size=BC,
                    queue_num=(g0 // GCH) % 4)
            vg4 = arena[:].rearrange("p (vt s b) -> p vt b s", vt=NVT, s=DV)
            obv = out_b[:].rearrange("p (vt b) -> p vt b", vt=NVT)
            nc.vector.tensor_reduce(obv, vg4, axis=AX.X, op=ALU.add)
            nc.vector.tensor_tensor(ovv, obv, xsb_vv, op=ALU.add)
            if it + 1 < N_ITER:
                nc.sync.dma_start(outr_flat, out_v[:])
            # transpose copy to batch-major and write the iteration output
            obb = out_b[:].rearrange("p (b vt) -> p vt b", b=BC)
            nc.vector.tensor_scalar_add(obb, ovv, 0.0)
            od_bv = out_d[it].rearrange("b (p vt) -> p b vt", p=128)
            nc.sync.dma_start(od_bv,
                              out_b[:].rearrange("p (b vt) -> p b vt", b=BC))

    nc.compile()
    return nc


def _numpy_fallback(llr, vi, ci):
    x = llr.T.astype(np.float32)
    scattered = x[vi]
    ext = np.zeros_like(scattered)
    outs = []
    for _ in range(N_ITER):
        vsum = np.zeros((N_VAR, x.shape[1]), np.float32)
        np.add.at(vsum, vi, ext)
        msg = (vsum[vi] - ext) + scattered
        t = np.tanh(msg * 0.5)
        la = np.log(np.abs(t) + EPS)
        sg = np.sign(t)
        cs = np.zeros((N_CHK, x.shape[1]), np.float32)
        np.add.at(cs, ci, la)
        cpr = np.ones((N_CHK, x.shape[1]), np.float32)
        np.multiply.at(cpr, ci, sg)
        loo = np.exp(cs[ci] - la) * (cpr[ci] * sg)
        loo = np.clip(loo, -float(_C), float(_C))
        ext = 2.0 * np.arctanh(loo)
        vs2 = np.zeros((N_VAR, x.shape[1]), np.float32)
        np.add.at(vs2, vi, ext)
        outs.append((vs2 + x).T)
    return np.stack(outs)


def kernel(llr, var_index, chk_index):
    llr = np.asarray(llr, np.float32)
    vi = np.asarray(var_index, np.int64).ravel()
    ci = np.asarray(chk_index, np.int64).ravel()
    assert llr.shape == (BATCH, N_VAR) and vi.shape == (E,) and ci.shape == (E,)

    regular = (np.array_equal(np.bincount(vi, minlength=N_VAR),
                              np.full(N_VAR, DV))
               and np.array_equal(np.bincount(ci, minlength=N_CHK),
                                  np.full(N_CHK, DC)))
    if not regular:
        return _numpy_fallback(llr, vi, ci).astype(np.float32)

    key = ("k3", hash(vi.tobytes()), hash(ci.tobytes()))
    if key not in _CACHE:
        planes = _build_indices(vi, ci)
        nc = _build_bass()
        _CACHE[key] = (nc, planes)
    nc, planes = _CACHE[key]

    from concourse.bass_utils import run_bass_kernel_spmd
    in_maps = []
    for c in range(N_CORES):
        m = {nm: np.ascontiguousarray(v) for nm, v in planes.items()}
        m["llr"] = np.ascontiguousarray(llr[c * BC:(c + 1) * BC, :])
        in_maps.append(m)
    trace = os.environ.get("BASS_KERNEL_TRACE", "0") == "1"
    res = run_bass_kernel_spmd(nc, in_maps, list(range(N_CORES)), trace=trace)
    global _LAST_RESULTS
    _LAST_RESULTS = res
    out = np.concatenate([res.results[c]["out"] for c in range(N_CORES)],
                         axis=1)
    return np.ascontiguousarray(out, dtype=np.float32)


if __name__ == "__main__":
    sys.path.insert(0, os.path.dirname(os.path.abspath(__file__)))
    import reference
    inputs = {k: np.asarray(v) for k, v in reference.setup_inputs().items()}
    exp = np.asarray(reference.reference(**inputs))
    got = kernel(**inputs)
    err = np.max(np.abs(got - exp)) / (np.max(np.abs(exp)) + 1e-30)
    print("Relative error:", err)


# revision 30
# speedup vs baseline: 4.5946x; 1.1040x over previous
"""Trainium2 Bass kernel for NeuralSumProductModel (LDPC sum-product decoder).

Contract: kernel(**inputs) takes FULL inputs (llr [512,8192] f32,
var_index [24576] i32, chk_index [24576] i32) and returns the FULL
output [5, 512, 8192] f32, matching reference.reference().

Design v3 (per NeuronCore, batch sharded 512 -> 8 x 64): batch-on-free
layout + dma_gather. ap_gather moves 4B per index (~27ns/idx); dma_gather
moves a 256B row (64 f32 = one batch row) per descriptor at ~0.34ns/desc
generation and DMA-bus execution, so all permutation traffic rides DMA.

  - SBUF check layout: partition p owns checks {ct*128+p : ct in [0,32)},
    cols [ct][e][b] (e in [0,6) edge slot, b in [0,64) batch).
  - msg_e = out_prev[var(e)] - ext_prev[e]: out rows live in DRAM OUTR
    [8192, 64]; dma_gather pulls row var(e) for each edge slot.
  - check phase: 4 pieces x 8 check-tiles (3072 cols), baseline numerics
    (tanh, sign, abs, ln, strided reduce6, phi involution, sign via
    reduce-mult). New ext written to SBUF EXTSB (positional reuse next
    iteration) and streamed to DRAM EXTR [24576, 64] rows r=p*192+ct*6+e.
  - var phase: dma_gather pulls ext rows at each var's 3 edge positions
    -> VG [p, vt, s, b]; vsum = reduce over s; out = vsum + x; written
    back to OUTR (v-major rows) and to out_d (batch-major) from a
    transposed copy.
"""

import os
import sys

import numpy as np

for _p in ("/opt/trn_rl_repo", "/root/.axon_site/_ro/trn_rl_repo"):
    if os.path.isdir(_p) and _p not in sys.path:
        sys.path.insert(0, _p)

N_VAR, N_CHK, DV, DC = 8192, 4096, 3, 6
E = N_VAR * DV  # 24576
BATCH, N_ITER, N_CORES = 512, 5, 8
BC = BATCH // N_CORES           # 64 batch rows per core
NCT = N_CHK // 128              # 32 check tiles
NVT = N_VAR // 128              # 64 var blocks per partition (v = p*64 + vt)
NP_CHK = 4                      # check pieces per iteration
CTP = NCT // NP_CHK             # 8 check tiles per piece
PW = CTP * DC * BC              # 3072 cols per piece
W = NCT * DC * BC               # 12288 cols total (ext per partition)

GCH = 512                       # dma_gather rows per call (SWDGE ring cap
                                # ~1024; 512 lets two calls overlap per queue)

EPS = 1e-12
_C = np.float32(1.0) - np.float32(1e-7)
TCLIP = float(np.float32((np.float32(1.0) - _C) / (np.float32(1.0) + _C)))

_CACHE = {}
_LAST_RESULTS = None


def _wrap(stream):
    """Pack an index stream [n] -> wrapped [128, n//16], replicated across
    the 8 gpsimd cores (dma_gather uses one shared stream)."""
    st = np.asarray(stream, np.int16)
    n = st.shape[0]
    assert n % 16 == 0
    core = st.reshape(n // 16, 16).T     # [16, n//16]
    return np.tile(core, (8, 1))


def _build_indices(vi, ci):
    """Host-side graph preprocessing. Returns dict of wrapped index planes."""
    order = np.argsort(ci, kind="stable")          # check-major edge list
    cm_var = vi[order].astype(np.int64)            # var of each cm edge
    pos_of_edge = np.empty(E, np.int64)
    pos_of_edge[order] = np.arange(E)
    edges_of_var = np.argsort(vi, kind="stable").reshape(N_VAR, DV)
    pos_var = pos_of_edge[edges_of_var]            # [N_VAR, 3] cm positions

    # msg gather: OUTR row = var id of edge slot (c = ct*128 + p, e);
    # output slot j = (ct_loc*6 + e)*128 + p within each piece.
    ixm = np.zeros(E, np.int64)
    for pc in range(NP_CHK):
        for jl in range(CTP * DC):
            ctl, e = jl // DC, jl % DC
            c = (pc * CTP + ctl) * 128 + np.arange(128)
            ixm[pc * CTP * DC * 128 + jl * 128:
                pc * CTP * DC * 128 + (jl + 1) * 128] = cm_var[c * DC + e]
    planes = {"ixm": _wrap(ixm)}

    # var gather: EXTR row of var v's s-th edge; v = p*64 + vt,
    # output slot j = (vt*3 + s)*128 + p.
    vidx = np.zeros(DV * N_VAR, np.int64)
    for vt in range(NVT):
        for s in range(DV):
            p = np.arange(128)
            v = p * NVT + vt
            j = pos_var[v, s]                      # cm position
            c, e = j // DC, j % DC
            r = (c % 128) * (NCT * DC) + (c // 128) * DC + e
            vidx[(vt * DV + s) * 128 + p] = r
    planes["vidx"] = _wrap(vidx)
    return planes


def _build_bass():
    import concourse.bass as bass
    import concourse.tile as tile
    from concourse import bacc, mybir
    from contextlib import ExitStack

    dt = mybir.dt
    F32, BF16, I16 = dt.float32, dt.bfloat16, dt.int16
    ALU = mybir.AluOpType
    ACT = mybir.ActivationFunctionType
    AX = mybir.AxisListType

    # 4 SWDGE queues: each dma_gather call's await_space blocks only on its
    # own ring, so desc-gen and DMA execution overlap across queues.
    nc = bacc.Bacc("TRN2", target_bir_lowering=False, debug=False,
                   num_swdge_queues=4)

    llr_d = nc.dram_tensor("llr", [BC, N_VAR], F32, kind="ExternalInput").ap()
    ixm_d = nc.dram_tensor("ixm", [128, E // 16], I16,
                           kind="ExternalInput").ap()
    vidx_d = nc.dram_tensor("vidx", [128, DV * N_VAR // 16], I16,
                            kind="ExternalInput").ap()
    out_d = nc.dram_tensor("out", [N_ITER, BC, N_VAR], F32,
                           kind="ExternalOutput").ap()
    outr = nc.dram_tensor("outr", [N_VAR, BC], F32, kind="Internal").ap()
    extr = nc.dram_tensor("extr", [E, BC], F32, kind="Internal").ap()

    outr_flat = outr[:, :].rearrange("(p n) k -> p (n k)", p=128)
    extr_flat = extr[:, :].rearrange("(p n) k -> p (n k)", p=128)
    llr_bv = llr_d[:, :].rearrange("b (p vt) -> p b vt", p=128)

    with tile.TileContext(nc) as tc, ExitStack() as ctx:
        big = ctx.enter_context(tc.tile_pool(name="big", bufs=1))

        arena = big.tile([128, W], F32, tag="arena")         # 48KB
        extsb = big.tile([128, W], F32, tag="extsb")         # 48KB
        sgt = big.tile([128, 2 * PW], BF16, tag="sgt")       # 12KB
        out_v = big.tile([128, N_VAR // 2], F32, tag="out_v")   # 16KB
        out_b = big.tile([128, N_VAR // 2], F32, tag="out_b")   # 16KB
        xsb = big.tile([128, N_VAR // 2], F32, tag="xsb")    # 16KB (b-major)
        xsb_v = big.tile([128, N_VAR // 2], F32, tag="xsb_v")  # x, v-major
        csum = big.tile([128, 2 * CTP * BC], F32, tag="csum")
        cpt = big.tile([128, 2 * CTP * BC], BF16, tag="cpt")
        scs = big.tile([128, 4 * CTP * BC], F32, tag="scs")
        epst = big.tile([128, 1], F32, tag="epst")
        ixm_t = big.tile([128, E // 16], I16, tag="ixm_t")
        vidx_t = big.tile([128, DV * N_VAR // 16], I16, tag="vidx_t")

        TA = [arena[:, 0:PW], arena[:, PW:2 * PW]]
        TB = [arena[:, 2 * PW:3 * PW], arena[:, 3 * PW:4 * PW]]
        SG = [sgt[:, 0:PW], sgt[:, PW:2 * PW]]
        CS = [csum[:, 0:CTP * BC], csum[:, CTP * BC:2 * CTP * BC]]
        CP = [cpt[:, 0:CTP * BC], cpt[:, CTP * BC:2 * CTP * BC]]

        nc.vector.memset(epst[:], EPS)
        nc.vector.memset(extsb[:], 0.0)
        nc.sync.dma_start(ixm_t[:], ixm_d[:])
        nc.sync.dma_start(vidx_t[:], vidx_d[:])
        nc.sync.dma_start(xsb[:].rearrange("p (b vt) -> p b vt", b=BC),
                          llr_bv)

        # x in v-major layout (one-time transpose copy), and OUTR := x rows
        xsb_vv = xsb[:].rearrange("p (b vt) -> p vt b", b=BC)
        xv_vv = xsb_v[:].rearrange("p (vt b) -> p vt b", vt=NVT)
        ovv = out_v[:].rearrange("p (vt b) -> p vt b", vt=NVT)
        nc.vector.tensor_scalar_add(xv_vv, xsb_vv, 0.0)
        nc.sync.dma_start(outr_flat, xsb_v[:])

        for it in range(N_ITER):
            for pc in range(NP_CHK):
                k = pc % 2
                cl = slice(pc * PW, (pc + 1) * PW)
                ta, tb, sg, cs, cp = TA[k], TB[k], SG[k], CS[k], CP[k]

                nip = CTP * DC * 128        # 6144 gathered rows per piece
                # SWDGE ring caps a single dma_gather at ~1024 descriptors
                # (larger calls wedge the device); chunk the gather.
                for g0 in range(0, nip, GCH):
                    dsl = ta[:, (g0 // 128) * BC:((g0 + GCH) // 128) * BC]
                    nc.gpsimd.dma_gather(
                        dsl.rearrange("p (n k) -> p n k", k=BC), outr[:, :],
                        ixm_t[:, (pc * nip + g0) // 16:
                              (pc * nip + g0 + GCH) // 16],
                        num_idxs=GCH, num_idxs_reg=GCH, elem_size=BC,
                        queue_num=(g0 // GCH) % 4)

                # msg = gather(out_prev) - ext_prev
                nc.vector.tensor_tensor(tb, ta, extsb[:, cl],
                                        op=ALU.subtract)
                nc.scalar.activation(ta, tb, ACT.Tanh, scale=0.5)
                nc.scalar.activation(sg, ta, ACT.Sign)
                nc.scalar.activation(tb, ta, ACT.Abs)
                nc.scalar.activation(ta, tb, ACT.Ln, bias=epst[:])

                # per-edge-slot slices are [p, ct, b] with contiguous b runs;
                # slice-wise ops avoid the ~3.5x DVE strided-reduce penalty.
                la_e = [ta.rearrange("p (ct e b) -> p ct e b", ct=CTP, e=DC)
                        [:, :, e, :] for e in range(DC)]
                sg_e = [sg.rearrange("p (ct e b) -> p ct e b", ct=CTP, e=DC)
                        [:, :, e, :] for e in range(DC)]
                cs6 = cs.rearrange("p (ct b) -> p ct b", ct=CTP)
                cp6 = cp.rearrange("p (ct b) -> p ct b", ct=CTP)
                SM = CTP * BC
                tA, tB, tC, tD = (scs[:, i * SM:(i + 1) * SM].rearrange(
                    "p (ct b) -> p ct b", ct=CTP) for i in range(4))
                # csum tree: ((e0+e1)+(e2+e3)) + (e4+e5), no aliasing
                nc.vector.tensor_tensor(tA, la_e[0], la_e[1], op=ALU.add)
                nc.vector.tensor_tensor(tB, la_e[2], la_e[3], op=ALU.add)
                nc.vector.tensor_tensor(tC, la_e[4], la_e[5], op=ALU.add)
                nc.vector.tensor_tensor(tD, tA, tB, op=ALU.add)
                nc.vector.tensor_tensor(cs6, tD, tC, op=ALU.add)
                # sign product tree
                nc.vector.tensor_tensor(tA, sg_e[0], sg_e[1], op=ALU.mult)
                nc.vector.tensor_tensor(tB, sg_e[2], sg_e[3], op=ALU.mult)
                nc.vector.tensor_tensor(tC, sg_e[4], sg_e[5], op=ALU.mult)
                nc.vector.tensor_tensor(tD, tA, tB, op=ALU.mult)
                nc.vector.tensor_tensor(cp6, tD, tC, op=ALU.mult)

                dd_e = [tb.rearrange("p (ct e b) -> p ct e b", ct=CTP, e=DC)
                        [:, :, e, :] for e in range(DC)]
                for e in range(DC):
                    nc.vector.tensor_tensor(dd_e[e], cs6, la_e[e],
                                            op=ALU.subtract)

                nc.scalar.activation(ta, tb, ACT.Tanh, scale=-0.5)
                nc.vector.tensor_scalar_max(tb, ta, TCLIP)
                nc.scalar.activation(ta, tb, ACT.Ln)

                se_e = dd_e
                for e in range(DC):
                    nc.vector.tensor_tensor(se_e[e], sg_e[e], cp6,
                                            op=ALU.mult)
                nc.vector.scalar_tensor_tensor(
                    extsb[:, cl], ta, -1.0, tb, op0=ALU.mult, op1=ALU.mult)

                nc.sync.dma_start(extr_flat[:, cl], extsb[:, cl])

            # ---- var phase ----
            for g0 in range(0, DV * N_VAR, GCH):
                dsl = arena[:, (g0 // 128) * BC:((g0 + GCH) // 128) * BC]
                nc.gpsimd.dma_gather(
                    dsl.rearrange("p (n k) -> p n k", k=BC), extr[:, :],
                    vidx_t[:, g0 // 16:(g0 + GCH) // 16],
                    num_idxs=GCH, num_idxs_reg=GCH, elem_size=BC,
                    queue_num=(g0 // GCH) % 4)
            # out = x + s0 + s1 + s2 via contiguous-output slice adds;
            # final V-layout lands in out_b's storage, B-layout in out_v's.
            vg_s = [arena[:].rearrange("p (vt s b) -> p vt s b",
                                       vt=NVT, s=DV)[:, :, s, :]
                    for s in range(DV)]
            obv = out_b[:].rearrange("p (vt b) -> p vt b", vt=NVT)
            nc.vector.tensor_tensor(obv, vg_s[0], xv_vv, op=ALU.add)
            nc.vector.tensor_tensor(ovv, obv, vg_s[1], op=ALU.add)
            nc.vector.tensor_tensor(obv, ovv, vg_s[2], op=ALU.add)
            if it + 1 < N_ITER:
                nc.sync.dma_start(outr_flat, out_b[:])
            # transpose copy to batch-major and write the iteration output
            ovb = out_v[:].rearrange("p (b vt) -> p vt b", b=BC)
            nc.vector.tensor_scalar_add(ovb, obv, 0.0)
            od_bv = out_d[it].rearrange("b (p vt) -> p b vt", p=128)
            nc.sync.dma_start(od_bv,
                              out_v[:].rearrange("p (b vt) -> p b vt", b=BC))

    nc.compile()
    return nc


def _numpy_fallback(llr, vi, ci):
    x = llr.T.astype(np.float32)
    scattered = x[vi]
    ext = np.zeros_like(scattered)
    outs = []
    for _ in range(N_ITER):
        vsum = np.zeros((N_VAR, x.shape[1]), np.float32)
        np.add.at(vsum, vi, ext)
        msg = (vsum[vi] - ext) + scattered
        t = np.tanh(msg * 0.5)
        la = np.log(np.abs(t) + EPS)
        sg = np.sign(t)
        cs = np.zeros((N_CHK, x.shape[1]), np.float32)
        np.add.at(cs, ci, la)
        cpr = np.ones((N_CHK, x.shape[1]), np.float32)
        np.multiply.at(cpr, ci, sg)
        loo = np.exp(cs[ci] - la) * (cpr[ci] * sg)
        loo = np.clip(loo, -float(_C), float(_C))
        ext = 2.0 * np.arctanh(loo)
        vs2 = np.zeros((N_VAR, x.shape[1]), np.float32)
        np.add.at(vs2, vi, ext)
        outs.append((vs2 + x).T)
    return np.stack(outs)


def kernel(llr, var_index, chk_index):
    llr = np.asarray(llr, np.float32)
    vi = np.asarray(var_index, np.int64).ravel()
    ci = np.asarray(chk_index, np.int64).ravel()
    assert llr.shape == (BATCH, N_VAR) and vi.shape == (E,) and ci.shape == (E,)

    regular = (np.array_equal(np.bincount(vi, minlength=N_VAR),
                              np.full(N_VAR, DV))
               and np.array_equal(np.bincount(ci, minlength=N_CHK),
                                  np.full(N_CHK, DC)))
    if not regular:
        return _numpy_fallback(llr, vi, ci).astype(np.float32)

    key = ("k3", hash(vi.tobytes()), hash(ci.tobytes()))
    if key not in _CACHE:
        planes = _build_indices(vi, ci)
        nc = _build_bass()
        _CACHE[key] = (nc, planes)
    nc, planes = _CACHE[key]

    from concourse.bass_utils import run_bass_kernel_spmd
    in_maps = []
    for c in range(N_CORES):
        m = {nm: np.ascontiguousarray(v) for nm, v in planes.items()}
        m["llr"] = np.ascontiguousarray(llr[c * BC:(c + 1) * BC, :])
        in_maps.append(m)
    trace = os.environ.get("BASS_KERNEL_TRACE", "0") == "1"
    res = run_bass_kernel_spmd(nc, in_maps, list(range(N_CORES)), trace=trace)
    global _LAST_RESULTS
    _LAST_RESULTS = res
    out = np.concatenate([res.results[c]["out"] for c in range(N_CORES)],
                         axis=1)
    return np.ascontiguousarray(out, dtype=np.float32)


if __name__ == "__main__":
    sys.path.insert(0, os.path.dirname(os.path.abspath(__file__)))
    import reference
    inputs = {k: np.asarray(v) for k, v in reference.setup_inputs().items()}
    exp = np.asarray(reference.reference(**inputs))
    got = kernel(**inputs)
    err = np.max(np.abs(got - exp)) / (np.max(np.abs(exp)) + 1e-30)
    print("Relative error:", err)
